# revision 31
# baseline (speedup 1.0000x reference)
"""Trainium2 Bass kernel for nn_CrossAttention (batch-parallel over 8 cores).

Reference computation (per batch element b):
    x   = proj_in(input)              # 1x1 conv -> [hw, emb]
    Q   = x @ wq ;  K = ctx @ wk ; V = ctx @ wv
    att = softmax(Q K^T * emb^-0.5)
    out = att @ V                     # [hw, emb]
    out = proj_out(concat([input, out], ch))   # 1x1 conv -> [in_ch, h, w]

Device strategy (data-parallel, one batch element per NeuronCore):
  * Same weight folds as the bf16 version: Wq_eff = proj_in^T wq * scale
    folded into G = Wq_eff K^T (computed on chip from H = wk Wq_eff^T),
    wv and the attention half of proj_out folded into VV = ctx (wv WoO).
  * The attention path is only ~1.3% of the output norm (the proj_out
    concat skip half WoA^T A dominates), so the whole attention pipeline
    runs in fp8e4 with DoubleRow matmuls (2 contraction k-tiles per PE
    pass): G, ST = G^T A, PT = exp, VV, and OT = VV^T PT_n.
  * Scale folding (fp8e4 max 240, subnormals below 2^-9):
        H host-scaled by 2^10  -> G8 = 1024*G  (evicted psum->fp8)
        A8 = fp8(A) unscaled   -> ST psum = 1024*ST ; exp scale 2^-10
        wv WoO host-scaled 2^6 -> VV8 = 64*VV
        PT_n = (PT * 2^13) * (1/sum PT)  (DVE scalar_tensor_tensor)
        WoA host-scaled by 2^19 (lossless bf16 shift)
    so OT psum = 64*8192*att = 2^19*att accumulates INTO THE SAME PSUM
    as the bf16 skip matmuls WoA19^T A -- one bank group holds
    2^19 * (att + WoA^T A); a single DVE tensor_scalar(2^-19) evicts
    the final output tile in bf16.  This fusion (instead of separate
    OT / OUT_A psums + multiply + add) is what fits 8 PSUM banks:
    st(3) + combined(4) + sum(1).
  * Per-pixel softmax denominators via a fp8 ones-matmul over PT
    (cross-partition sum broadcast to 128 partitions), DVE fast
    reciprocal.
  * Per block PE stream: ST(16 DR) -> skip OA(16 bf16, opens the psum
    groups, covers the exp tail on ScalarE) -> sum(4 DR) -> OT(16 DR,
    closes the groups).  DVE: recip, 8x PT_n, 4x evict.  ScalarE: 8x exp.
  * Inputs/outputs host-retiled so every DMA is a contiguous burst,
    spread over sync/gpsimd/vector DGE queues.  Dummy matmuls warm the
    PE HAM clock-gate while the first DMAs are in flight.
"""

import numpy as np
import ml_dtypes

import concourse.bass as bass
import concourse.tile as tile
from concourse import bacc, mybir
from concourse.bass_utils import run_bass_kernel_spmd

BF16 = mybir.dt.bfloat16
F8 = mybir.dt.float8e4
F32 = mybir.dt.float32
DR = mybir.MatmulPerfMode.DoubleRow

C = 512      # in channels
E = 512      # emb dim
HW = 4096    # 64*64 image positions
L = 1024     # 32*32 context positions
P = 128      # partitions
B = 512      # positions per block
NBLK = HW // B    # 8
CT_T = C // P     # 4  tiles of input channels
ET = E // P       # 4  tiles of emb features
LT = L // P       # 8  tiles of context positions

G_SCALE = 2.0 ** -10    # exp() descale for the 1024x G fold
PT_SCALE = 2.0 ** 13    # PT_n pre-normalization scale
OUT_SCALE = 2.0 ** -19  # final evict descale (= 64 * 8192)


def build_kernel():
    nc = bacc.Bacc("TRN2", target_bir_lowering=False, debug=False,
                   num_devices=8, enable_asserts=False)

    a_d = nc.dram_tensor("a", [NBLK, CT_T, P, B], BF16, kind="ExternalInput")
    a8_d = nc.dram_tensor("a8", [NBLK, CT_T, P, B], F8, kind="ExternalInput")
    # cw = concat([ctx, 1024 * wk Wq_eff^T], free dim): one DMA per
    # partition-tile covers everything G needs -> minimal startup latency.
    cw_d = nc.dram_tensor("cw", [E, L + C], F8, kind="ExternalInput")
    wv_d = nc.dram_tensor("wv", [E, C], F8, kind="ExternalInput")   # 64 * wv WoO
    wo_d = nc.dram_tensor("wo", [C, C], BF16, kind="ExternalInput")  # 2^19 * WoA
    out_d = nc.dram_tensor("out", [NBLK, CT_T, P, B], BF16,
                           kind="ExternalOutput")

    # partition-major views of the DRAM tensors: [p, tile, free]
    cw_v = cw_d.ap().rearrange("(t p) f -> p t f", p=P)
    wv_v = wv_d.ap().rearrange("(t p) f -> p t f", p=P)
    wo_v = wo_d.ap().rearrange("(t p) f -> p t f", p=P)

    with tile.TileContext(nc) as tc:
        with (
            tc.tile_pool(name="const", bufs=1) as const,
            tc.tile_pool(name="ablk", bufs=2) as a_pool,
            tc.tile_pool(name="a8blk", bufs=2) as a8_pool,
            tc.tile_pool(name="pt", bufs=2) as pt_pool,
            tc.tile_pool(name="ptn", bufs=2) as ptn_pool,
            tc.tile_pool(name="osb", bufs=4) as out_pool,
            tc.tile_pool(name="rb", bufs=1) as rb_pool,
            tc.tile_pool(name="stps", bufs=3, space="PSUM") as st_psum,
            tc.tile_pool(name="cmps", bufs=4, space="PSUM") as comb_psum,
            tc.tile_pool(name="smps", bufs=1, space="PSUM") as sm_psum,
        ):
            qs = [nc.sync, nc.gpsimd, nc.scalar]
            lqs = [nc.sync, nc.gpsimd]

            def spread_dma(dst, src, n, off=0):
                for k in range(n):
                    qs[(k + off) % len(qs)].dma_start(
                        out=dst[:, k], in_=src[:, k])

            # PE warm-up: dummy matmuls while the first input DMAs are in
            # flight (HAM clock-gate ramp).  memset on GpSimd: its queue
            # comes up first, so the PE starts ~2us earlier than with a
            # DVE-produced tile.
            warm = const.tile([P, B], BF16)
            nc.gpsimd.memset(warm, 1.0)
            wps = sm_psum.tile([P, B], F32, tag="small")
            for _ in range(10):
                nc.tensor.matmul(wps, warm[:, 0:P], warm, start=True,
                                 stop=True)
            warm_guard = const.tile([1, 1], F32)
            nc.vector.tensor_copy(out=warm_guard, in_=wps[0:1, 0:1])

            # startup-critical load first: ctx + H(wq) in one tensor,
            # one DMA per partition-tile spread over the three queues.
            cw_sb = const.tile([P, ET, L + C], F8)
            spread_dma(cw_sb, cw_v, ET)
            ct_sb = cw_sb[:, :, 0:L]
            wq_sb = cw_sb[:, :, L:L + C]

            def load_a(ib):
                blk8 = a8_pool.tile([P, CT_T, B], F8, tag="a8")
                blk = a_pool.tile([P, CT_T, B], BF16, tag="a")
                lqs[ib % 2].dma_start(
                    out=blk8, in_=a8_d.ap()[ib].rearrange("k p f -> p k f"))
                lqs[(ib + 1) % 2].dma_start(
                    out=blk, in_=a_d.ap()[ib].rearrange("k p f -> p k f"))
                return blk8, blk

            wv_sb = const.tile([P, ET, C], F8)
            spread_dma(wv_sb, wv_v, ET, off=2)
            blk0 = load_a(0)
            wo_sb = const.tile([P, CT_T, C], BF16)
            spread_dma(wo_sb, wo_v, CT_T, off=1)
            ones_mat = const.tile([P, 2, P], F8)
            nc.gpsimd.memset(ones_mat, 1.0)

            # ---- G = H^T ctx = 1024 * Wq_eff K^T  [C, L] (fp8 DR) -----
            g_sb = const.tile([P, CT_T, L], F8)
            for m in range(CT_T):
                for n2 in range(L // B):
                    ps = st_psum.tile([P, B], F32, tag="mm")
                    for k in range(ET // 2):
                        nc.tensor.matmul(
                            ps,
                            wq_sb[:, 2 * k:2 * k + 2, m * P:(m + 1) * P],
                            ct_sb[:, 2 * k:2 * k + 2, n2 * B:(n2 + 1) * B],
                            start=(k == 0),
                            stop=(k == ET // 2 - 1),
                            perf_mode=DR,
                        )
                    nc.vector.tensor_copy(
                        out=g_sb[:, m, n2 * B:(n2 + 1) * B], in_=ps)

            # ---- VV = ctx (wv WoO) * 64   [L, C] (fp8 DR) -------------
            v_sb = const.tile([P, LT, C], F8)
            for mj in range(LT):
                ps = comb_psum.tile([P, C], F32, tag="mm")
                for k in range(ET // 2):
                    nc.tensor.matmul(
                        ps,
                        ct_sb[:, 2 * k:2 * k + 2, mj * P:(mj + 1) * P],
                        wv_sb[:, 2 * k:2 * k + 2, :],
                        start=(k == 0),
                        stop=(k == ET // 2 - 1),
                        perf_mode=DR,
                    )
                nc.vector.tensor_copy(out=v_sb[:, mj, :], in_=ps)

            # ---- per block of B positions -----------------------------
            def block(ib, a8_blk, a_blk):
                # ST = G^T A (fp8 DR), PT = exp(ST * 2^-10) in fp8
                pt_blk = pt_pool.tile([P, LT, B], F8, tag="pt")
                for mj in range(LT):
                    ps = st_psum.tile([P, B], F32, tag="mm")
                    for k in range(CT_T // 2):
                        nc.tensor.matmul(
                            ps,
                            g_sb[:, 2 * k:2 * k + 2, mj * P:(mj + 1) * P],
                            a8_blk[:, 2 * k:2 * k + 2, :],
                            start=(k == 0),
                            stop=(k == CT_T // 2 - 1),
                            perf_mode=DR,
                        )
                    nc.scalar.activation(
                        out=pt_blk[:, mj, :], in_=ps,
                        func=mybir.ActivationFunctionType.Exp,
                        scale=G_SCALE,
                    )

                nxt = load_a(ib + 1) if ib + 1 < NBLK else None

                # skip path: open the combined psum groups with
                # 2^19 * WoA^T A (bf16); covers the exp tail on ScalarE.
                comb_ps = []
                for mo in range(CT_T):
                    ps = comb_psum.tile([P, B], F32, tag="mm")
                    for kc in range(CT_T):
                        nc.tensor.matmul(
                            ps,
                            wo_sb[:, kc, mo * P:(mo + 1) * P],
                            a_blk[:, kc, :],
                            start=(kc == 0),
                            stop=False,
                        )
                    comb_ps.append(ps)

                # denominators: ones-matmul over PT (fp8 DR), bcast sum
                b_ps = sm_psum.tile([P, B], F32, tag="small")
                for k in range(LT // 2):
                    nc.tensor.matmul(
                        b_ps, ones_mat, pt_blk[:, 2 * k:2 * k + 2, :],
                        start=(k == 0), stop=(k == LT // 2 - 1),
                        perf_mode=DR,
                    )
                rb_sb = rb_pool.tile([P, B], F32, tag="rb")
                nc.vector.reciprocal_approx_fast(out=rb_sb, in_=b_ps)

                # PT_n = (PT * 2^13) * (1/sum)  in fp8  (DVE)
                ptn_blk = ptn_pool.tile([P, LT, B], F8, tag="ptn")
                for mj in range(LT):
                    nc.vector.scalar_tensor_tensor(
                        out=ptn_blk[:, mj, :], in0=pt_blk[:, mj, :],
                        scalar=PT_SCALE, in1=rb_sb,
                        op0=mybir.AluOpType.mult, op1=mybir.AluOpType.mult,
                    )

                # OT = VV8^T PT_n (fp8 DR) accumulated into the skip psum.
                # k-major order: each freshly produced PT_n pair feeds 4
                # PE passes, so the DVE producer is never on the critical
                # path.  Last block goes mo-major instead: psum group 0
                # completes early so its evict+DMA overlap the remaining
                # groups (shorter drain tail).
                last = ib == NBLK - 1
                if last:
                    order = [(k, mo) for mo in range(CT_T)
                             for k in range(LT // 2)]
                else:
                    order = [(k, mo) for k in range(LT // 2)
                             for mo in range(CT_T)]
                for k, mo in order:
                    nc.tensor.matmul(
                        comb_ps[mo],
                        v_sb[:, 2 * k:2 * k + 2, mo * P:(mo + 1) * P],
                        ptn_blk[:, 2 * k:2 * k + 2, :],
                        start=False, stop=(k == LT // 2 - 1),
                        perf_mode=DR,
                    )

                # evict: out = psum * 2^-19 in bf16 -> DRAM, split over
                # DVE and ScalarE so psum banks free in parallel.
                oqs = qs if last else lqs
                for mo in range(CT_T):
                    o_sb = out_pool.tile([P, B], BF16, tag="osb")
                    if mo % 2 == 1:
                        nc.scalar.activation(
                            out=o_sb, in_=comb_ps[mo],
                            func=mybir.ActivationFunctionType.Copy,
                            scale=OUT_SCALE,
                        )
                    else:
                        nc.vector.tensor_scalar_mul(o_sb, comb_ps[mo],
                                                    OUT_SCALE)
                    oqs[mo % len(oqs)].dma_start(
                        out=out_d.ap()[ib, mo], in_=o_sb)
                return nxt

            blk = blk0
            for ib in range(NBLK):
                blk = block(ib, *blk)

    nc.compile()
    return nc


_NC = None


def _get_nc():
    global _NC
    if _NC is None:
        _NC = build_kernel()
    return _NC


def run(inputs: dict, trace: bool = False):
    """Shard inputs over 8 cores, run the SPMD kernel, gather the output."""
    bf = ml_dtypes.bfloat16
    f8 = ml_dtypes.float8_e4m3
    inp = np.asarray(inputs["input"], np.float32).reshape(8, C, HW)
    ctx = np.asarray(inputs["context"], np.float32).reshape(8, E, L)
    proj_in_w = np.asarray(inputs["proj_in_w"], np.float32)
    wq_w = np.asarray(inputs["wq_w"], np.float32)
    wk_w = np.asarray(inputs["wk_w"], np.float32)
    wv_w = np.asarray(inputs["wv_w"], np.float32)
    proj_out_w = np.asarray(inputs["proj_out_w"], np.float32)

    scale = float(E) ** -0.5
    wq_eff = (proj_in_w.T @ wq_w) * scale        # [C, E]
    h_w = np.ascontiguousarray(wk_w @ wq_eff.T) * 1024.0         # [E, C]
    wo_full = proj_out_w.T                        # [C+E, C]
    w_vo = np.ascontiguousarray(wv_w @ wo_full[C:]) * 64.0       # [E, C]
    wo_a = np.ascontiguousarray(wo_full[:C]) * (2.0 ** 19)       # [C, C]

    def to_f8(x):
        return np.clip(x, -240.0, 240.0).astype(f8)

    # block-tiled, fully contiguous per-DMA layout [blk, ctile, p, f]
    a_tiled = np.ascontiguousarray(
        inp.reshape(8, CT_T, P, NBLK, B).transpose(0, 3, 1, 2, 4))
    a_all = a_tiled.astype(bf)
    a8_all = to_f8(a_tiled)
    h8 = to_f8(h_w)

    in_maps = [
        {
            "a": a_all[i],
            "a8": a8_all[i],
            "cw": np.concatenate([to_f8(ctx[i]), h8], axis=1),
            "wv": to_f8(w_vo),
            "wo": wo_a.astype(bf),
        }
        for i in range(8)
    ]

    nc = _get_nc()
    res = run_bass_kernel_spmd(nc, in_maps, core_ids=list(range(8)), trace=trace)
    out = np.stack([np.asarray(res.results[i]["out"]) for i in range(8)])
    # [8, blk, ctile, p, f] -> [8, C, HW]
    out = out.astype(np.float32).transpose(0, 2, 3, 1, 4).reshape(8, C, 64, 64)
    return np.ascontiguousarray(out), res


def kernel(**inputs) -> np.ndarray:
    out, _ = run(inputs, trace=False)
    return out


# revision 33
# speedup vs baseline: 1.0002x; 1.0002x over previous
"""Trainium2 Bass kernel for nn_CrossAttention (batch-parallel over 8 cores).

Reference computation (per batch element b):
    x   = proj_in(input)              # 1x1 conv -> [hw, emb]
    Q   = x @ wq ;  K = ctx @ wk ; V = ctx @ wv
    att = softmax(Q K^T * emb^-0.5)
    out = att @ V                     # [hw, emb]
    out = proj_out(concat([input, out], ch))   # 1x1 conv -> [in_ch, h, w]

Device strategy (data-parallel, one batch element per NeuronCore):
  * Same weight folds as the bf16 version: Wq_eff = proj_in^T wq * scale
    folded into G = Wq_eff K^T (computed on chip from H = wk Wq_eff^T),
    wv and the attention half of proj_out folded into VV = ctx (wv WoO).
  * The attention path is only ~1.3% of the output norm (the proj_out
    concat skip half WoA^T A dominates), so the whole attention pipeline
    runs in fp8e4 with DoubleRow matmuls (2 contraction k-tiles per PE
    pass): G, ST = G^T A, PT = exp, VV, and OT = VV^T PT_n.
  * Scale folding (fp8e4 max 240, subnormals below 2^-9):
        H host-scaled by 2^10  -> G8 = 1024*G  (evicted psum->fp8)
        A8 = fp8(A) unscaled   -> ST psum = 1024*ST ; exp scale 2^-10
        wv WoO host-scaled 2^6 -> VV8 = 64*VV
        PT_n = (PT * 2^13) * (1/sum PT)  (DVE scalar_tensor_tensor)
        WoA host-scaled by 2^19 (lossless bf16 shift)
    so OT psum = 64*8192*att = 2^19*att accumulates INTO THE SAME PSUM
    as the bf16 skip matmuls WoA19^T A -- one bank group holds
    2^19 * (att + WoA^T A); a single DVE tensor_scalar(2^-19) evicts
    the final output tile in bf16.  This fusion (instead of separate
    OT / OUT_A psums + multiply + add) is what fits 8 PSUM banks:
    st(3) + combined(4) + sum(1).
  * Per-pixel softmax denominators via a fp8 ones-matmul over PT
    (cross-partition sum broadcast to 128 partitions), DVE fast
    reciprocal.
  * Per block PE stream: ST(16 DR) -> skip OA(16 bf16, opens the psum
    groups, covers the exp tail on ScalarE) -> sum(4 DR) -> OT(16 DR,
    closes the groups).  OT is issued k-major so each PT_n pair from the
    DVE feeds 4 back-to-back PE passes (producer never critical); the
    last block goes mo-major + evicts split DVE/ScalarE + 3 DMA queues
    so the drain tail overlaps the remaining matmuls.  Measured: every
    512-col pass retires in 216ns (2.4 GHz, fp8 DR = 2x bf16 MACs/pass);
    the kernel sits at ~97% of that envelope.
  * Startup: ctx and H ship as ONE concatenated fp8 tensor (one DMA per
    partition-tile, 3 queues) and 13 dummy matmuls on a GpSimd-memset
    tile bridge the PE HAM clock ramp until those land -- any PE idle
    here drops the clock to mid-pstate and costs ~6us on G/VV.
  * Inputs/outputs host-retiled so every DMA is a contiguous burst.
"""

import numpy as np
import ml_dtypes

import concourse.bass as bass
import concourse.tile as tile
from concourse import bacc, mybir
from concourse.bass_utils import run_bass_kernel_spmd

BF16 = mybir.dt.bfloat16
F8 = mybir.dt.float8e4
F32 = mybir.dt.float32
DR = mybir.MatmulPerfMode.DoubleRow

C = 512      # in channels
E = 512      # emb dim
HW = 4096    # 64*64 image positions
L = 1024     # 32*32 context positions
P = 128      # partitions
B = 512      # positions per block
NBLK = HW // B    # 8
CT_T = C // P     # 4  tiles of input channels
ET = E // P       # 4  tiles of emb features
LT = L // P       # 8  tiles of context positions

G_SCALE = 2.0 ** -10    # exp() descale for the 1024x G fold
PT_SCALE = 2.0 ** 13    # PT_n pre-normalization scale
OUT_SCALE = 2.0 ** -19  # final evict descale (= 64 * 8192)


def build_kernel():
    nc = bacc.Bacc("TRN2", target_bir_lowering=False, debug=False,
                   num_devices=8, enable_asserts=False)

    a_d = nc.dram_tensor("a", [NBLK, CT_T, P, B], BF16, kind="ExternalInput")
    a8_d = nc.dram_tensor("a8", [NBLK, CT_T, P, B], F8, kind="ExternalInput")
    # cw = concat([ctx, 1024 * wk Wq_eff^T], free dim): one DMA per
    # partition-tile covers everything G needs -> minimal startup latency.
    cw_d = nc.dram_tensor("cw", [E, L + C], F8, kind="ExternalInput")
    wv_d = nc.dram_tensor("wv", [E, C], F8, kind="ExternalInput")   # 64 * wv WoO
    wo_d = nc.dram_tensor("wo", [C, C], BF16, kind="ExternalInput")  # 2^19 * WoA
    out_d = nc.dram_tensor("out", [NBLK, CT_T, P, B], BF16,
                           kind="ExternalOutput")

    # partition-major views of the DRAM tensors: [p, tile, free]
    cw_v = cw_d.ap().rearrange("(t p) f -> p t f", p=P)
    wv_v = wv_d.ap().rearrange("(t p) f -> p t f", p=P)
    wo_v = wo_d.ap().rearrange("(t p) f -> p t f", p=P)

    with tile.TileContext(nc) as tc:
        with (
            tc.tile_pool(name="const", bufs=1) as const,
            tc.tile_pool(name="ablk", bufs=2) as a_pool,
            tc.tile_pool(name="a8blk", bufs=2) as a8_pool,
            tc.tile_pool(name="pt", bufs=2) as pt_pool,
            tc.tile_pool(name="ptn", bufs=2) as ptn_pool,
            tc.tile_pool(name="osb", bufs=4) as out_pool,
            tc.tile_pool(name="rb", bufs=1) as rb_pool,
            tc.tile_pool(name="stps", bufs=3, space="PSUM") as st_psum,
            tc.tile_pool(name="cmps", bufs=4, space="PSUM") as comb_psum,
            tc.tile_pool(name="smps", bufs=1, space="PSUM") as sm_psum,
        ):
            qs = [nc.sync, nc.gpsimd, nc.scalar]
            lqs = [nc.sync, nc.gpsimd]

            def spread_dma(dst, src, n, off=0):
                for k in range(n):
                    qs[(k + off) % len(qs)].dma_start(
                        out=dst[:, k], in_=src[:, k])

            # PE warm-up: dummy matmuls while the first input DMAs are in
            # flight (HAM clock-gate ramp).  memset on GpSimd: its queue
            # comes up first, so the PE starts ~2us earlier than with a
            # DVE-produced tile.
            warm = const.tile([P, B], BF16)
            nc.gpsimd.memset(warm, 1.0)
            wps = sm_psum.tile([P, B], F32, tag="small")
            for _ in range(13):
                nc.tensor.matmul(wps, warm[:, 0:P], warm, start=True,
                                 stop=True)
            warm_guard = const.tile([1, 1], F32)
            nc.vector.tensor_copy(out=warm_guard, in_=wps[0:1, 0:1])

            # startup-critical load first: ctx + H(wq) in one tensor,
            # one DMA per partition-tile spread over the three queues.
            cw_sb = const.tile([P, ET, L + C], F8)
            spread_dma(cw_sb, cw_v, ET)
            ct_sb = cw_sb[:, :, 0:L]
            wq_sb = cw_sb[:, :, L:L + C]

            def load_a(ib):
                blk8 = a8_pool.tile([P, CT_T, B], F8, tag="a8")
                blk = a_pool.tile([P, CT_T, B], BF16, tag="a")
                lqs[ib % 2].dma_start(
                    out=blk8, in_=a8_d.ap()[ib].rearrange("k p f -> p k f"))
                lqs[(ib + 1) % 2].dma_start(
                    out=blk, in_=a_d.ap()[ib].rearrange("k p f -> p k f"))
                return blk8, blk

            wv_sb = const.tile([P, ET, C], F8)
            spread_dma(wv_sb, wv_v, ET, off=2)
            blk0 = load_a(0)
            wo_sb = const.tile([P, CT_T, C], BF16)
            spread_dma(wo_sb, wo_v, CT_T, off=1)
            ones_mat = const.tile([P, 2, P], F8)
            nc.gpsimd.memset(ones_mat, 1.0)

            # ---- G = H^T ctx = 1024 * Wq_eff K^T  [C, L] (fp8 DR) -----
            g_sb = const.tile([P, CT_T, L], F8)
            for m in range(CT_T):
                for n2 in range(L // B):
                    ps = st_psum.tile([P, B], F32, tag="mm")
                    for k in range(ET // 2):
                        nc.tensor.matmul(
                            ps,
                            wq_sb[:, 2 * k:2 * k + 2, m * P:(m + 1) * P],
                            ct_sb[:, 2 * k:2 * k + 2, n2 * B:(n2 + 1) * B],
                            start=(k == 0),
                            stop=(k == ET // 2 - 1),
                            perf_mode=DR,
                        )
                    nc.vector.tensor_copy(
                        out=g_sb[:, m, n2 * B:(n2 + 1) * B], in_=ps)

            # ---- VV = ctx (wv WoO) * 64   [L, C] (fp8 DR) -------------
            v_sb = const.tile([P, LT, C], F8)
            for mj in range(LT):
                ps = comb_psum.tile([P, C], F32, tag="mm")
                for k in range(ET // 2):
                    nc.tensor.matmul(
                        ps,
                        ct_sb[:, 2 * k:2 * k + 2, mj * P:(mj + 1) * P],
                        wv_sb[:, 2 * k:2 * k + 2, :],
                        start=(k == 0),
                        stop=(k == ET // 2 - 1),
                        perf_mode=DR,
                    )
                nc.vector.tensor_copy(out=v_sb[:, mj, :], in_=ps)

            # ---- per block of B positions -----------------------------
            def block(ib, a8_blk, a_blk):
                # ST = G^T A (fp8 DR), PT = exp(ST * 2^-10) in fp8
                pt_blk = pt_pool.tile([P, LT, B], F8, tag="pt")
                for mj in range(LT):
                    ps = st_psum.tile([P, B], F32, tag="mm")
                    for k in range(CT_T // 2):
                        nc.tensor.matmul(
                            ps,
                            g_sb[:, 2 * k:2 * k + 2, mj * P:(mj + 1) * P],
                            a8_blk[:, 2 * k:2 * k + 2, :],
                            start=(k == 0),
                            stop=(k == CT_T // 2 - 1),
                            perf_mode=DR,
                        )
                    nc.scalar.activation(
                        out=pt_blk[:, mj, :], in_=ps,
                        func=mybir.ActivationFunctionType.Exp,
                        scale=G_SCALE,
                    )

                nxt = load_a(ib + 1) if ib + 1 < NBLK else None

                # skip path: open the combined psum groups with
                # 2^19 * WoA^T A (bf16); covers the exp tail on ScalarE.
                comb_ps = []
                for mo in range(CT_T):
                    ps = comb_psum.tile([P, B], F32, tag="mm")
                    for kc in range(CT_T):
                        nc.tensor.matmul(
                            ps,
                            wo_sb[:, kc, mo * P:(mo + 1) * P],
                            a_blk[:, kc, :],
                            start=(kc == 0),
                            stop=False,
                        )
                    comb_ps.append(ps)

                # denominators: ones-matmul over PT (fp8 DR), bcast sum
                b_ps = sm_psum.tile([P, B], F32, tag="small")
                for k in range(LT // 2):
                    nc.tensor.matmul(
                        b_ps, ones_mat, pt_blk[:, 2 * k:2 * k + 2, :],
                        start=(k == 0), stop=(k == LT // 2 - 1),
                        perf_mode=DR,
                    )
                rb_sb = rb_pool.tile([P, B], F32, tag="rb")
                nc.vector.reciprocal_approx_fast(out=rb_sb, in_=b_ps)

                # PT_n = (PT * 2^13) * (1/sum)  in fp8  (DVE)
                ptn_blk = ptn_pool.tile([P, LT, B], F8, tag="ptn")
                for mj in range(LT):
                    nc.vector.scalar_tensor_tensor(
                        out=ptn_blk[:, mj, :], in0=pt_blk[:, mj, :],
                        scalar=PT_SCALE, in1=rb_sb,
                        op0=mybir.AluOpType.mult, op1=mybir.AluOpType.mult,
                    )

                # OT = VV8^T PT_n (fp8 DR) accumulated into the skip psum.
                # k-major order: each freshly produced PT_n pair feeds 4
                # PE passes, so the DVE producer is never on the critical
                # path.  Last block goes mo-major instead: psum group 0
                # completes early so its evict+DMA overlap the remaining
                # groups (shorter drain tail).
                last = ib == NBLK - 1
                if last:
                    order = [(k, mo) for mo in range(CT_T)
                             for k in range(LT // 2)]
                else:
                    order = [(k, mo) for k in range(LT // 2)
                             for mo in range(CT_T)]
                for k, mo in order:
                    nc.tensor.matmul(
                        comb_ps[mo],
                        v_sb[:, 2 * k:2 * k + 2, mo * P:(mo + 1) * P],
                        ptn_blk[:, 2 * k:2 * k + 2, :],
                        start=False, stop=(k == LT // 2 - 1),
                        perf_mode=DR,
                    )

                # evict: out = psum * 2^-19 in bf16 -> DRAM, split over
                # DVE and ScalarE so psum banks free in parallel.
                oqs = qs if last else lqs
                for mo in range(CT_T):
                    o_sb = out_pool.tile([P, B], BF16, tag="osb")
                    if mo % 2 == 1:
                        nc.scalar.activation(
                            out=o_sb, in_=comb_ps[mo],
                            func=mybir.ActivationFunctionType.Copy,
                            scale=OUT_SCALE,
                        )
                    else:
                        nc.vector.tensor_scalar_mul(o_sb, comb_ps[mo],
                                                    OUT_SCALE)
                    oqs[mo % len(oqs)].dma_start(
                        out=out_d.ap()[ib, mo], in_=o_sb)
                return nxt

            blk = blk0
            for ib in range(NBLK):
                blk = block(ib, *blk)

    nc.compile()
    return nc


_NC = None


def _get_nc():
    global _NC
    if _NC is None:
        _NC = build_kernel()
    return _NC


def run(inputs: dict, trace: bool = False):
    """Shard inputs over 8 cores, run the SPMD kernel, gather the output."""
    bf = ml_dtypes.bfloat16
    f8 = ml_dtypes.float8_e4m3
    inp = np.asarray(inputs["input"], np.float32).reshape(8, C, HW)
    ctx = np.asarray(inputs["context"], np.float32).reshape(8, E, L)
    proj_in_w = np.asarray(inputs["proj_in_w"], np.float32)
    wq_w = np.asarray(inputs["wq_w"], np.float32)
    wk_w = np.asarray(inputs["wk_w"], np.float32)
    wv_w = np.asarray(inputs["wv_w"], np.float32)
    proj_out_w = np.asarray(inputs["proj_out_w"], np.float32)

    scale = float(E) ** -0.5
    wq_eff = (proj_in_w.T @ wq_w) * scale        # [C, E]
    h_w = np.ascontiguousarray(wk_w @ wq_eff.T) * 1024.0         # [E, C]
    wo_full = proj_out_w.T                        # [C+E, C]
    w_vo = np.ascontiguousarray(wv_w @ wo_full[C:]) * 64.0       # [E, C]
    wo_a = np.ascontiguousarray(wo_full[:C]) * (2.0 ** 19)       # [C, C]

    def to_f8(x):
        return np.clip(x, -240.0, 240.0).astype(f8)

    # block-tiled, fully contiguous per-DMA layout [blk, ctile, p, f]
    a_tiled = np.ascontiguousarray(
        inp.reshape(8, CT_T, P, NBLK, B).transpose(0, 3, 1, 2, 4))
    a_all = a_tiled.astype(bf)
    a8_all = to_f8(a_tiled)
    h8 = to_f8(h_w)

    in_maps = [
        {
            "a": a_all[i],
            "a8": a8_all[i],
            "cw": np.concatenate([to_f8(ctx[i]), h8], axis=1),
            "wv": to_f8(w_vo),
            "wo": wo_a.astype(bf),
        }
        for i in range(8)
    ]

    nc = _get_nc()
    res = run_bass_kernel_spmd(nc, in_maps, core_ids=list(range(8)), trace=trace)
    out = np.stack([np.asarray(res.results[i]["out"]) for i in range(8)])
    # [8, blk, ctile, p, f] -> [8, C, HW]
    out = out.astype(np.float32).transpose(0, 2, 3, 1, 4).reshape(8, C, 64, 64)
    return np.ascontiguousarray(out), res


def kernel(**inputs) -> np.ndarray:
    out, _ = run(inputs, trace=False)
    return out
